# revision 1
# baseline (speedup 1.0000x reference)
"""Trainium2 Bass kernel for 3D multi-head attention (nn_Attention3D).

Problem: x [1, 16, 16, 16, 528] -> full attention over N=4096 tokens,
8 heads of dim 66, qkv + out projections.

Sharding: one head per NeuronCore (8 cores). Each core computes its
head's q/k/v projections, full 4096x4096 attention, and its partial
contribution to the output projection. Host sums the 8 partials and
adds the output bias.

Key layout decisions (all host-side prep, free):
  - x is pre-transposed on host to xT [640, 4096] (C on partitions),
    with row 528 = 1.0 (bias row) and rows 529-639 = 0 padding. This
    makes every on-device matmul contraction sit on the partition dim
    with K=128 chunks, with qkv biases folded into the weight matmuls.
  - q is pre-scaled by hd^-0.5 (folded into wq/bq on host).
  - v gets an extra ones-column (col 66), so the attention-value
    matmul also accumulates the softmax denominator for free.
  - Scores are computed transposed ([k-tokens, q-tokens]) so softmax's
    sum runs over the partition dim via the ones-column trick, exp runs
    on ScalarE straight out of PSUM, and no transposes are ever needed.
  - Attention-path matmul operands (x, qkv weights, qT/kT, exp(scores),
    v) are bfloat16 -- the PE's native 1-cycle/row dtype with fast
    weight load; PSUM accumulation is always fp32. The final projection
    (oT @ wp) stays float32r (fp32-class precision). Measured end to
    end: ~255us/core, rel err ~1.5e-3 vs the fp32 reference (fp16 runs
    at 2 cycles/row on TRN2; all-f32r is ~307us at 1.3e-4 if more
    accuracy is ever needed). float32r requires f32r-typed producers
    and even innermost AP sizes (hence the 68-wide v tile).
"""

import numpy as np

import ml_dtypes

BF16_NP = ml_dtypes.bfloat16

EMBED = 528
HD = 66
NHEADS = 8
NT = 4096
NCH = 5  # contraction chunks of 128 (640 = 528 + bias row + pad)
G = 3  # k-chunks per exp group (3 PSUM banks per scores tile)


def _build_nc(nt=NT):
    import concourse.tile as tile
    from concourse import bacc, mybir

    F32 = mybir.dt.float32
    F32R = mybir.dt.float32r  # fast fp32 matmul mode
    BF16 = mybir.dt.bfloat16  # attention operands: true 1 cyc/row + FWL
    AF = mybir.ActivationFunctionType

    nkc = nt // 128  # k-token chunks
    nqb = nt // 512  # q-token blocks
    ntb = nt // 128  # token blocks for the projection

    nc = bacc.Bacc("TRN2", target_bir_lowering=False, debug=False)
    xT_d = nc.dram_tensor("xT", [NCH, 128, nt], BF16, kind="ExternalInput").ap()
    wq_d = nc.dram_tensor("wq", [128, NCH, 128], BF16, kind="ExternalInput").ap()
    wk_d = nc.dram_tensor("wk", [128, NCH, 128], BF16, kind="ExternalInput").ap()
    z_d = nc.dram_tensor("zeros", [128, nt], F32R, kind="ExternalInput").ap()
    wv_d = nc.dram_tensor("wv", [128, NCH, HD + 2], BF16, kind="ExternalInput").ap()
    wp_d = nc.dram_tensor("wp", [128, EMBED], F32R, kind="ExternalInput").ap()
    y_d = nc.dram_tensor("y", [nt, EMBED], F32, kind="ExternalOutput").ap()

    with tile.TileContext(nc) as tc:
        with (
            tc.tile_pool(name="const", bufs=1) as constp,
            tc.tile_pool(name="persist", bufs=1) as pp,
        ):
            wq = constp.tile([128, NCH, 128], BF16, name="wq_sb")
            wk = constp.tile([128, NCH, 128], BF16, name="wk_sb")
            wv = constp.tile([128, NCH, HD + 2], BF16, name="wv_sb")
            wp = constp.tile([128, EMBED], F32R, name="wp_sb")
            nc.sync.dma_start(wq[:], wq_d[:])
            nc.sync.dma_start(wk[:], wk_d[:])

            # qT/kT/oT are hd-padded to 128 partitions (rows HD.. stay 0) so
            # every matmul contracts over a full K=128.
            qT = pp.tile([128, nt], BF16, name="qT")
            kT = pp.tile([128, nt], BF16, name="kT")
            oT = pp.tile([128, nt], F32R, name="oT")
            vaug = pp.tile([128, nkc, HD + 2], BF16, name="vaug")
            recipT = pp.tile([128, ntb], F32, name="recipT")

            # ---------------- Phase A: qkv projections ----------------
            with (
                tc.tile_pool(name="xp", bufs=1) as xp,
                tc.tile_pool(name="psA", bufs=4, space="PSUM") as psA,
            ):
                xT = xp.tile([128, NCH, nt], BF16, name="xT_sb")
                # chunked DMA so compute can start before the full 10MB lands
                for b in range(nqb):
                    qs = slice(b * 512, (b + 1) * 512)
                    for c in range(NCH):
                        nc.sync.dma_start(xT[:, c, qs], xT_d[c, :, qs])
                # late-needed loads, emitted after x so compute starts sooner:
                # wv before the v pass, zeros (oT rows 67-127 must be zero for
                # the projection matmul) before phase B's oT copies, wp before
                # the first projection.
                nc.sync.dma_start(wv[:], wv_d[:])
                nc.sync.dma_start(oT[:], z_d[:])
                nc.sync.dma_start(wp[:], wp_d[:])

                # interleave the q and k accumulation chains (independent
                # PSUM banks) so consecutive PE matmuls can pipeline instead
                # of running at isolated fill+drain latency.
                for b in range(nqb):
                    qs = slice(b * 512, (b + 1) * 512)
                    ps_q = psA.tile([128, 512], F32, tag="qk", name="ps_q")
                    ps_k = psA.tile([128, 512], F32, tag="qk", name="ps_k")
                    for c in range(NCH):
                        for w, ps in ((wq, ps_q), (wk, ps_k)):
                            nc.tensor.matmul(
                                ps[:],
                                w[:, c, :],
                                xT[:, c, qs],
                                start=(c == 0),
                                stop=(c == NCH - 1),
                            )
                    nc.vector.tensor_copy(qT[:, qs], ps_q[:])
                    nc.vector.tensor_copy(kT[:, qs], ps_k[:])
                # same trick for v: two token-block chains in flight
                for t0 in range(0, nkc, 2):
                    psvs = [
                        psA.tile([128, HD + 2], F32, tag="v", name="ps_v")
                        for _ in range(2)
                    ]
                    for c in range(NCH):
                        for i in range(2):
                            ts_ = slice((t0 + i) * 128, (t0 + i + 1) * 128)
                            nc.tensor.matmul(
                                psvs[i][:],
                                xT[:, c, ts_],
                                wv[:, c, :],
                                start=(c == 0),
                                stop=(c == NCH - 1),
                            )
                    for i in range(2):
                        nc.vector.tensor_copy(vaug[:, t0 + i, :], psvs[i][:])

            # ---------------- Phase B: attention + projection ----------------
            # alternating 4/3-chunk exp groups double-buffered across two
            # PSUM pools (4+3 banks) + 1 bank for the oT accumulator = 8.
            groups = []
            kc0 = 0
            want = 4
            while kc0 < nkc:
                gsz = min(want, nkc - kc0)
                groups.append((kc0, gsz))
                kc0 += gsz
                want = 3 if want == 4 else 4

            with (
                tc.tile_pool(name="ep", bufs=3) as ep,
                tc.tile_pool(name="yp", bufs=3) as yp,
                tc.tile_pool(name="rp", bufs=2) as rp,
                tc.tile_pool(name="drp", bufs=2, space="DRAM") as drp,
                tc.tile_pool(name="psSa", bufs=1, space="PSUM") as psSa,
                tc.tile_pool(name="psSb", bufs=1, space="PSUM") as psSb,
                tc.tile_pool(name="psO", bufs=1, space="PSUM") as psO,
            ):
                for b in range(nqb):
                    qs = slice(b * 512, (b + 1) * 512)
                    o_ps = psO.tile([HD + 2, 512], F32, name="o_ps")

                    def emit_av(g0, gsz, E):
                        for j in range(gsz):
                            kc = g0 + j
                            nc.tensor.matmul(
                                o_ps[:],
                                vaug[:, kc, :],
                                E[:, j * 512 : (j + 1) * 512],
                                start=(kc == 0),
                                stop=(kc == nkc - 1),
                                skip_group_check=True,
                            )

                    # software pipeline: AV of group g-1 is emitted after the
                    # scores+exp of group g, so the PE streams scores(g) while
                    # ScalarE still exps group g-1 instead of stalling on it.
                    pending = None
                    for gi, (g0, gsz) in enumerate(groups):
                        if gi % 2 == 0:
                            sc = psSa.tile([128, 4 * 512], F32, tag="sca", name="sca")
                        else:
                            sc = psSb.tile([128, 3 * 512], F32, tag="scb", name="scb")
                        for j in range(gsz):
                            kc = g0 + j
                            nc.tensor.matmul(
                                sc[:, j * 512 : (j + 1) * 512],
                                kT[:, kc * 128 : (kc + 1) * 128],
                                qT[:, qs],
                                start=True,
                                stop=True,
                            )
                        E = ep.tile([128, 4 * 512], BF16, tag="E", name="E")
                        nc.scalar.activation(
                            E[:, : gsz * 512], sc[:, : gsz * 512], AF.Exp
                        )
                        if pending is not None:
                            emit_av(*pending)
                        pending = (g0, gsz, E)
                    emit_av(*pending)
                    recip = rp.tile([1, 512], F32, name="recip")
                    nc.vector.reciprocal_approx_fast(recip[:], o_ps[0:1, :])
                    dstage = drp.tile([1, 512], F32, name="dstage")
                    nc.sync.dma_start(dstage[:], recip[:])
                    nc.sync.dma_start(
                        recipT[:, b * 4 : (b + 1) * 4],
                        dstage.rearrange("o (f p) -> (o p) f", p=128),
                    )
                    nc.vector.tensor_copy(oT[: HD + 2, qs], o_ps[:])

            # ---------------- Phase C: output projection ----------------
            with (
                tc.tile_pool(name="yp", bufs=3) as yp,
                tc.tile_pool(name="psY", bufs=3, space="PSUM") as psY,
            ):
                for t in range(ntb):
                    ts_ = slice(t * 128, (t + 1) * 128)
                    yps = psY.tile([128, 1024], F32, name="yps")
                    nc.tensor.matmul(
                        yps[:, :512],
                        oT[:, ts_],
                        wp[:, :512],
                        start=True,
                        stop=True,
                    )
                    nc.tensor.matmul(
                        yps[:, 512 : 512 + (EMBED - 512)],
                        oT[:, ts_],
                        wp[:, 512:],
                        start=True,
                        stop=True,
                    )
                    ysb = yp.tile([128, EMBED], F32, tag="ysb", name="ysb")
                    if t % 2 == 0:
                        nc.vector.tensor_scalar_mul(
                            ysb[:], yps[:, :EMBED], recipT[:, t : t + 1]
                        )
                    else:
                        nc.scalar.activation(
                            ysb[:],
                            yps[:, :EMBED],
                            AF.Copy,
                            scale=recipT[:, t : t + 1],
                        )
                    nc.sync.dma_start(y_d[ts_, :], ysb[:])

    nc.compile()
    return nc


def _prep_inputs(x, w_qkv, b_qkv, w_proj, nt):
    """Host-side shard prep: returns list of 8 in_maps."""
    x = np.asarray(x, dtype=np.float32)
    w_qkv = np.asarray(w_qkv, dtype=np.float32)
    b_qkv = np.asarray(b_qkv, dtype=np.float32)
    w_proj = np.asarray(w_proj, dtype=np.float32)

    xt = x.reshape(nt, EMBED)
    xT_pad = np.zeros((NCH * 128, nt), dtype=np.float32)
    xT_pad[:EMBED] = xt.T
    xT_pad[EMBED] = 1.0
    xT_in = np.ascontiguousarray(xT_pad.reshape(NCH, 128, nt))

    s = float(HD) ** -0.5
    in_maps = []
    for h in range(NHEADS):
        sl_q = slice(h * HD, (h + 1) * HD)
        sl_k = slice(EMBED + h * HD, EMBED + (h + 1) * HD)
        sl_v = slice(2 * EMBED + h * HD, 2 * EMBED + (h + 1) * HD)

        wq_t = np.zeros((NCH * 128, 128), dtype=np.float32)
        wq_t[:EMBED, :HD] = (w_qkv[sl_q] * s).T
        wq_t[EMBED, :HD] = b_qkv[sl_q] * s

        wk_t = np.zeros((NCH * 128, 128), dtype=np.float32)
        wk_t[:EMBED, :HD] = w_qkv[sl_k].T
        wk_t[EMBED, :HD] = b_qkv[sl_k]

        # ones column sits at index 0 so the softmax denominator lands on
        # PSUM partition 0 (engine partition bases must be 32-aligned)
        # fp32r matmuls need even innermost sizes -> pad to 68 columns
        wv_t = np.zeros((NCH * 128, HD + 2), dtype=np.float32)
        wv_t[:EMBED, 1 : HD + 1] = w_qkv[sl_v].T
        wv_t[EMBED, 1 : HD + 1] = b_qkv[sl_v]
        wv_t[EMBED, 0] = 1.0  # ones column -> softmax denominator

        wp_t = np.zeros((128, EMBED), dtype=np.float32)
        wp_t[1 : HD + 1] = w_proj[:, sl_q].T  # row 0 = 0 kills the denom row

        in_maps.append(
            {
                "xT": xT_in.astype(BF16_NP),
                "wq": np.ascontiguousarray(
                    wq_t.reshape(NCH, 128, 128).transpose(1, 0, 2)
                ).astype(BF16_NP),
                "wk": np.ascontiguousarray(
                    wk_t.reshape(NCH, 128, 128).transpose(1, 0, 2)
                ).astype(BF16_NP),
                "zeros": np.zeros((128, nt), dtype=np.float32),
                "wv": np.ascontiguousarray(
                    wv_t.reshape(NCH, 128, HD + 2).transpose(1, 0, 2)
                ).astype(BF16_NP),
                "wp": wp_t,
            }
        )
    return in_maps


_NC_CACHE = {}


def _get_nc(nt=NT):
    if nt not in _NC_CACHE:
        _NC_CACHE[nt] = _build_nc(nt)
    return _NC_CACHE[nt]


def kernel(x, w_qkv, b_qkv, w_proj, b_proj, _trace=False):
    from concourse.bass_utils import run_bass_kernel_spmd

    x = np.asarray(x, dtype=np.float32)
    b_proj = np.asarray(b_proj, dtype=np.float32)
    B, D, H, W, C = x.shape
    nt = D * H * W

    nc = _get_nc(nt)
    in_maps = _prep_inputs(x, w_qkv, b_qkv, w_proj, nt)
    res = run_bass_kernel_spmd(
        nc, in_maps, core_ids=list(range(NHEADS)), trace=_trace
    )
    out = np.zeros((nt, EMBED), dtype=np.float32)
    for r in res.results:
        out += r["y"]
    out += b_proj
    kernel.last_results = res
    return out.reshape(B, D, H, W, C)

